# revision 8
# baseline (speedup 1.0000x reference)
"""Trainium2 Bass kernel for nn_KANLayer: 8-core batch-sharded SPMD.

KAN layer: B-spline (k=3, 5 intervals, uniform grid) + SiLU base path.
Outputs (out, preacts, postacts, postspline) as in the reference.

Strategy
- Shard batch 4096 -> 8 cores x 512 rows. Per core, 4 chunks of 128 rows
  on SBUF partitions.
- Host folds mask*scale_sp into coef and packs per-input-group (q = 16
  input dims) weight matrices so the spline contraction, the SiLU base
  term, and the final sum over input dims are all TensorEngine matmuls.
- DVE computes exact Cox-de Boor bases (same op order/rounding as the
  reference, support-pruned to the nonzero j range).
- preacts is written by a broadcast-AP DMA straight from the x tile.
"""

import sys

sys.path.insert(0, "/opt/trn_rl_repo")

import numpy as np

import concourse.bass as bass
import concourse.tile as tile
from concourse import mybir
from concourse.bass_utils import run_bass_kernel_spmd
from concourse.masks import make_identity
from concourse.vector_clock import VectorClock
from concourse.tile_scheduler import N_PROCS
import bass_rust as _bass_rust

F32 = mybir.dt.float32
ALU = mybir.AluOpType

IN_DIM = 64
OUT_DIM = 64
NUM = 5
K = 3
SIZE = IN_DIM * OUT_DIM
BATCH = 4096
NCORES = 8
BSH = BATCH // NCORES          # 512 batch rows per core
NCH = BSH // 128               # 4 chunks of 128 rows
NI = NCH * IN_DIM              # 256 = free width of the x tile
NQ = 4                         # input-dim groups of 16
QW = IN_DIM // NQ              # 16
NC_COEF = NUM + K              # 8 basis channels


def _patch_drain(chunk=1):
    # This container's walrus allows only ONE sync wait per Drain
    # (TPB_CTRL) instruction; split the TileContext tail-drain waits
    # across several drains.
    def _drain_and_barrier(self, tick_clock, wait_clock):
        gc = tick_clock.global_clock
        vals = [gc[p] for p in range(N_PROCS)]
        nonzero = [p for p, v in enumerate(vals) if v > 0]
        for i in range(0, len(nonzero), chunk):
            group = set(nonzero[i : i + chunk])
            partial = VectorClock(
                [vals[p] if p in group else 0 for p in range(N_PROCS)]
            )
            d = self.nc.sync.drain()
            wait_clock.add_sem_waits(
                d.ins, _bass_rust.ScopedClock({None: partial})
            )
        self.nc.all_engine_barrier()
        popped = self.nc._tile_sem_poison_stack.pop()
        assert popped is self._sem_poison
        self.nc.clear_and_free_semaphores(list(self.sems.allocated().values()))
        self.nc.all_engine_barrier()

    tile.TileContext._drain_and_barrier = _drain_and_barrier


_patch_drain()


def _split_waits_json(bir_json: bytes, limit: int = 1) -> bytes:
    """This walrus build allows only `limit` sync waits per instruction.
    Hoist excess waits onto injected EventSemaphore instructions that run
    immediately before the instruction on the same engine."""
    import json

    d = json.loads(bir_json)
    counter = [0]

    def fix_block(b):
        insts = b.get("instructions", [])
        out = []
        for ins in insts:
            si = ins.get("sync_info") or {}
            waits = si.get("on_wait") or []
            if len(waits) > limit:
                keep = waits[-limit:]
                extra = waits[:-limit]
                for w in extra:
                    counter[0] += 1
                    out.append(
                        {
                            "debug": ins.get("debug"),
                            "engine": ins["engine"],
                            "ins": [],
                            "name": f"wsplit_{counter[0]}",
                            "opcode": "EventSemaphore",
                            "outs": [],
                            "sync_info": {"on_update": [], "on_wait": [w]},
                        }
                    )
                si["on_wait"] = keep
            out.append(ins)
        b["instructions"] = out
        for sub in b.get("blocks") or []:
            fix_block(sub)

    for f in d["functions"]:
        for b in f["blocks"]:
            fix_block(b)
    return json.dumps(d).encode()


def _patch_compile():
    from concourse import bass_utils as _bu
    from concourse import bass2jax as _b2j

    if getattr(_bu.compile_bir_kernel, "_wsplit", False):
        return
    _orig = _bu.compile_bir_kernel

    def patched(bir_json, tmpdir, neff_name="file.neff"):
        return _orig(_split_waits_json(bir_json), tmpdir, neff_name)

    patched._wsplit = True
    _bu.compile_bir_kernel = patched
    _b2j.compile_bir_kernel = patched


_patch_compile()


def _knots_from_grid(grid_row):
    """Replicate the reference's f32 grid extension exactly."""
    g = grid_row.astype(np.float32)
    h = np.float32((g[-1] - g[0]) / np.float32(g.shape[0] - 1))
    for _ in range(K):
        g = np.concatenate([g[:1] - h, g, g[-1:] + h]).astype(np.float32)
    return g  # 12 knots


def _lr_consts(g):
    """Per-level (d=1..3) left/right affine constants, f32.

    left_j(x)  = (x - g[j]) * invl[d][j]
    right_j(x) = (x - g[j+d+1]) * ninvr[d][j]   (= (g[j+d+1]-x)/denom)
    """
    one = np.float32(1.0)
    invl, ninvr, tl, tr = {}, {}, {}, {}
    for d in range(1, K + 1):
        nj = len(g) - d - 1
        invl[d] = np.array(
            [one / np.float32(g[j + d] - g[j]) for j in range(nj)], np.float32
        )
        ninvr[d] = np.array(
            [-(one / np.float32(g[j + d + 1] - g[j + 1])) for j in range(nj)],
            np.float32,
        )
        tl[d] = g[:nj].copy()
        tr[d] = g[d + 1 : d + 1 + nj].copy()
    return tl, invl, tr, ninvr


def _build_program():
    nc = bass.Bass("TRN2")

    xs = nc.dram_tensor("xs", [128, NCH, IN_DIM], F32, kind="ExternalInput")
    wraw = nc.dram_tensor("wraw", [NQ, 128, 1024], F32, kind="ExternalInput")
    wsc = nc.dram_tensor("wsc", [NQ, 128, 1024], F32, kind="ExternalInput")
    wb = nc.dram_tensor("wb", [NQ, QW, 1024], F32, kind="ExternalInput")
    wosp = nc.dram_tensor("wosp", [NQ, 128, OUT_DIM], F32, kind="ExternalInput")
    wob = nc.dram_tensor("wob", [NQ, QW, OUT_DIM], F32, kind="ExternalInput")
    kn = nc.dram_tensor("kn", [1, 64], F32, kind="ExternalInput")  # knot consts

    out_d = nc.dram_tensor("out", [NCH, 128, OUT_DIM], F32, kind="ExternalOutput")
    pre_d = nc.dram_tensor("pre", [NCH, 128, SIZE], F32, kind="ExternalOutput")
    pa_d = nc.dram_tensor("pa", [NCH, 128, SIZE], F32, kind="ExternalOutput")
    psp_d = nc.dram_tensor("psp", [NCH, 128, SIZE], F32, kind="ExternalOutput")

    # knot constants baked on host; slot layout in kn:
    #   [0:12] knots, then per level d: tl, invl, tr, ninvr packed later.
    # (scalars are compiled as immediates; kn kept for debugging only)

    with tile.TileContext(nc) as tc:
        with (
            tc.tile_pool(name="persist", bufs=1) as persist,
            tc.tile_pool(name="lr", bufs=2) as lrpool,
            tc.tile_pool(name="bt", bufs=8) as btpool,
            tc.tile_pool(name="osb", bufs=2) as osb,
            tc.tile_pool(name="psmm", bufs=5, space="PSUM") as psmm,
            tc.tile_pool(name="pstr", bufs=2, space="PSUM") as pstr,
            tc.tile_pool(name="psout", bufs=1, space="PSUM") as psout,
        ):
            ident = persist.tile([128, 128], F32)
            make_identity(nc, ident)

            x_t = persist.tile([128, NCH, IN_DIM], F32)
            nc.sync.dma_start(out=x_t[:], in_=xs[:, :, :])

            # preacts: pure broadcast of x over the out_dim axis.
            for n in range(NCH):
                nc.gpsimd.dma_start(
                    out=pre_d[n],
                    in_=x_t[:, n, None, :].broadcast_to([128, OUT_DIM, IN_DIM]),
                )

            # ---- weight loads ----
            w_raw_s = persist.tile([128, NQ, 1024], F32)
            w_sc_s = persist.tile([128, NQ, 1024], F32)
            w_b_s = persist.tile([QW, NQ, 1024], F32)
            w_osp_s = persist.tile([128, NQ, OUT_DIM], F32)
            w_ob_s = persist.tile([QW, NQ, OUT_DIM], F32)
            for q in range(NQ):
                nc.sync.dma_start(out=w_raw_s[:, q, :], in_=wraw[q])
                nc.sync.dma_start(out=w_sc_s[:, q, :], in_=wsc[q])
                nc.sync.dma_start(out=w_b_s[:, q, :], in_=wb[q])
                nc.sync.dma_start(out=w_osp_s[:, q, :], in_=wosp[q])
                nc.sync.dma_start(out=w_ob_s[:, q, :], in_=wob[q])

            # ---- exact Cox-de Boor bases on [128, NCH*IN_DIM] ----
            g = _build_program.knots
            tl, invl, tr, ninvr = _build_program.lr

            # j/c channel INNERMOST so per-(n,q) transpose slices are
            # contiguous [128, 16*8] (matmul RHS wants one free dim).
            xflat = x_t[:].rearrange("p n i -> p (n i)")

            ge = persist.tile([128, NI, 6], F32)
            for jj, j in enumerate(range(3, 9)):
                nc.vector.tensor_scalar(
                    out=ge[:, :, jj], in0=xflat,
                    scalar1=float(g[j]), scalar2=None, op0=ALU.is_ge,
                )
            b0 = persist.tile([128, NI, 5], F32)  # j = 3..7
            nc.vector.tensor_sub(b0[:], ge[:, :, 0:5], ge[:, :, 1:6])

            def level(bprev, jlo_prev, jlo, njout, d):
                """bprev holds B^{d-1} for j = jlo_prev .. ; returns B^d tile
                for j = jlo .. jlo+njout-1 (support-pruned)."""
                bnew = persist.tile([128, NI, njout], F32, tag=f"b{d}")
                lbuf = lrpool.tile([128, NI, njout], F32, tag="lbuf")
                rbuf = lrpool.tile([128, NI, njout], F32, tag="rbuf")
                for k in range(njout):
                    j = jlo + k
                    if k > 0:  # left factor needed (B^{d-1}_j nonzero)
                        nc.vector.tensor_scalar(
                            out=lbuf[:, :, k], in0=xflat,
                            scalar1=float(tl[d][j]), scalar2=float(invl[d][j]),
                            op0=ALU.subtract, op1=ALU.mult,
                        )
                    if k < njout - 1:  # right factor
                        nc.vector.tensor_scalar(
                            out=rbuf[:, :, k], in0=xflat,
                            scalar1=float(tr[d][j]), scalar2=float(ninvr[d][j]),
                            op0=ALU.subtract, op1=ALU.mult,
                        )
                # edge j = jlo: right-only; edge j = jlo+njout-1: left-only
                nc.vector.tensor_mul(bnew[:, :, 0], rbuf[:, :, 0], bprev[:, :, 0])
                nc.vector.tensor_mul(
                    bnew[:, :, njout - 1],
                    lbuf[:, :, njout - 1],
                    bprev[:, :, njout - 2],
                )
                ni_ = njout - 2
                nc.vector.tensor_mul(
                    lbuf[:, :, 1 : 1 + ni_],
                    lbuf[:, :, 1 : 1 + ni_],
                    bprev[:, :, 0:ni_],
                )
                nc.vector.tensor_mul(
                    rbuf[:, :, 1 : 1 + ni_],
                    rbuf[:, :, 1 : 1 + ni_],
                    bprev[:, :, 1 : 1 + ni_],
                )
                nc.vector.tensor_add(
                    bnew[:, :, 1 : 1 + ni_],
                    lbuf[:, :, 1 : 1 + ni_],
                    rbuf[:, :, 1 : 1 + ni_],
                )
                return bnew

            b1 = level(b0, 3, 2, 6, 1)   # j = 2..7
            b2 = level(b1, 2, 1, 7, 2)   # j = 1..7
            b3 = level(b2, 1, 0, 8, 3)   # j = 0..7 -> [128, (n i), 8]

            b3v = b3[:].rearrange("p (n i) c -> p n i c", n=NCH)

            # silu(x)^T, one [16, 128] tile per (q, n) so every matmul lhsT
            # starts at partition base 0 (PE quadrant rule).
            silu_sb = persist.tile([QW, NQ, NCH, 128], F32)

            out_sb_pool = osb
            for n in range(NCH):
                for q in range(NQ):
                    xt_ps = pstr.tile([QW, 128], F32, tag="tr")
                    nc.tensor.transpose(
                        xt_ps[:], x_t[:, n, q * QW : (q + 1) * QW], ident[:]
                    )
                    nc.scalar.activation(
                        out=silu_sb[:, q, n, :], in_=xt_ps[:],
                        func=mybir.ActivationFunctionType.Silu,
                    )

            for n in range(NCH):
                # transpose bases: per q, [128b, (ii c)] -> [(ii c), 128b]
                bts = []
                for q in range(NQ):
                    src = b3v[:, n, q * QW : (q + 1) * QW, :]
                    bt_ps = pstr.tile([128, 128], F32, tag="tr")
                    nc.tensor.transpose(bt_ps[:], src, ident[:])
                    bt_sb = btpool.tile([128, 128], F32, tag="btsb")
                    nc.scalar.copy(out=bt_sb[:], in_=bt_ps[:])
                    bts.append(bt_sb)

                psp_sb = osb.tile([128, OUT_DIM, IN_DIM], F32, tag="psp")
                pa_sb = osb.tile([128, OUT_DIM, IN_DIM], F32, tag="pa")
                out_ps = psout.tile([128, OUT_DIM], F32, tag="outps")

                eng_i = 0
                for q in range(NQ):
                    for h in range(2):
                        cols = slice(h * 512, (h + 1) * 512)
                        osl = slice(32 * h, 32 * h + 32)
                        qsl = slice(q * QW, (q + 1) * QW)
                        # postspline
                        mm = psmm.tile([128, 32, QW], F32, tag="mm")
                        nc.tensor.matmul(
                            mm[:], bts[q][:], w_raw_s[:, q, cols],
                            start=True, stop=True,
                        )
                        # postacts = scaled spline + silu base
                        mm2 = psmm.tile([128, 32, QW], F32, tag="mm")
                        nc.tensor.matmul(
                            mm2[:], bts[q][:], w_sc_s[:, q, cols],
                            start=True, stop=False,
                        )
                        nc.tensor.matmul(
                            mm2[:], silu_sb[:, q, n, :], w_b_s[:, q, cols],
                            start=False, stop=True,
                        )
                        if eng_i % 2 == 0:
                            nc.scalar.copy(out=psp_sb[:, osl, qsl], in_=mm[:])
                            nc.vector.tensor_copy(out=pa_sb[:, osl, qsl], in_=mm2[:])
                        else:
                            nc.vector.tensor_copy(out=psp_sb[:, osl, qsl], in_=mm[:])
                            nc.scalar.copy(out=pa_sb[:, osl, qsl], in_=mm2[:])
                        eng_i += 1

                # out = sum_i postacts: dense per-q matmuls, accumulate
                for q in range(NQ):
                    qsl = slice(q * QW, (q + 1) * QW)
                    nc.tensor.matmul(
                        out_ps[:], bts[q][:], w_osp_s[:, q, :],
                        start=(q == 0), stop=False,
                    )
                    nc.tensor.matmul(
                        out_ps[:], silu_sb[:, q, n, :], w_ob_s[:, q, :],
                        start=False, stop=(q == NQ - 1),
                    )
                o_sb = out_sb_pool.tile([128, OUT_DIM], F32, tag="osb")
                nc.scalar.copy(out=o_sb[:], in_=out_ps[:])

                nc.sync.dma_start(
                    out=psp_d[n], in_=psp_sb[:].rearrange("p a b -> p (a b)")
                )
                nc.scalar.dma_start(
                    out=pa_d[n], in_=pa_sb[:].rearrange("p a b -> p (a b)")
                )
                nc.sync.dma_start(out=out_d[n], in_=o_sb[:])

    return nc


def _prep_host(x, grid, coef, scale_base, scale_sp, mask):
    x = np.asarray(x, np.float32)
    grid = np.asarray(grid, np.float32)
    coef = np.asarray(coef, np.float32)
    scale_base = np.asarray(scale_base, np.float32)
    scale_sp = np.asarray(scale_sp, np.float32)
    mask = np.asarray(mask, np.float32)

    g = _knots_from_grid(grid[0])
    _build_program.knots = g
    _build_program.lr = _lr_consts(g)

    mss = (mask * scale_sp).astype(np.float32)
    msb = (mask * scale_base).astype(np.float32)
    coef_sc = (coef * mss[:, None]).astype(np.float32)

    coef_r = coef.reshape(OUT_DIM, IN_DIM, NC_COEF)
    coef_sc_r = coef_sc.reshape(OUT_DIM, IN_DIM, NC_COEF)
    msb_r = msb.reshape(OUT_DIM, IN_DIM)

    wraw = np.zeros((NQ, QW, NC_COEF, OUT_DIM, QW), np.float32)
    wsc = np.zeros_like(wraw)
    wb = np.zeros((NQ, QW, OUT_DIM, QW), np.float32)
    wosp = np.zeros((NQ, QW, NC_COEF, OUT_DIM), np.float32)
    wob = np.zeros((NQ, QW, OUT_DIM), np.float32)
    for q in range(NQ):
        for ii in range(QW):
            i = q * QW + ii
            wraw[q, ii, :, :, ii] = coef_r[:, i, :].T
            wsc[q, ii, :, :, ii] = coef_sc_r[:, i, :].T
            wb[q, ii, :, ii] = msb_r[:, i]
            wosp[q, ii, :, :] = coef_sc_r[:, i, :].T
            wob[q, ii, :] = msb_r[:, i]
    wraw = wraw.reshape(NQ, 128, 1024)
    wsc = wsc.reshape(NQ, 128, 1024)
    wb = wb.reshape(NQ, QW, 1024)
    wosp = wosp.reshape(NQ, 128, OUT_DIM)
    wob = wob.reshape(NQ, QW, OUT_DIM)

    kn = np.zeros((1, 64), np.float32)
    kn[0, :12] = g

    in_maps = []
    for c in range(NCORES):
        xsh = x[c * BSH : (c + 1) * BSH]  # [512, 64]
        xst = np.ascontiguousarray(
            xsh.reshape(NCH, 128, IN_DIM).transpose(1, 0, 2)
        )  # [128, NCH, 64]
        in_maps.append(
            dict(xs=xst, wraw=wraw, wsc=wsc, wb=wb, wosp=wosp, wob=wob, kn=kn)
        )
    return in_maps


_PROGRAM_CACHE = {}


def kernel(x, grid, coef, scale_base, scale_sp, mask, _trace=False):
    in_maps = _prep_host(x, grid, coef, scale_base, scale_sp, mask)
    key = "prog"
    if key not in _PROGRAM_CACHE:
        _PROGRAM_CACHE[key] = _build_program()
    nc = _PROGRAM_CACHE[key]

    res = run_bass_kernel_spmd(
        nc, in_maps, core_ids=list(range(NCORES)), trace=_trace
    )
    kernel.last_result = res

    outs, pres, pas, psps = [], [], [], []
    for c in range(NCORES):
        r = res.results[c]
        # [NCH, 128, ...] with b_local = n*128 + p -> reshape direct
        outs.append(r["out"].reshape(BSH, OUT_DIM))
        pres.append(r["pre"].reshape(BSH, OUT_DIM, IN_DIM))
        pas.append(r["pa"].reshape(BSH, OUT_DIM, IN_DIM))
        psps.append(r["psp"].reshape(BSH, OUT_DIM, IN_DIM))
    out = np.concatenate(outs, 0)
    preacts = np.concatenate(pres, 0)
    postacts = np.concatenate(pas, 0)
    postspline = np.concatenate(psps, 0)
    return (out, preacts, postacts, postspline)


# revision 9
# speedup vs baseline: 1.0601x; 1.0601x over previous
"""Trainium2 Bass kernel for nn_KANLayer: 8-core batch-sharded SPMD.

KAN layer: B-spline (k=3, 5 intervals, uniform grid) + SiLU base path.
Outputs (out, preacts, postacts, postspline) as in the reference.

Strategy
- Shard batch 4096 -> 8 cores x 512 rows. Per core, 4 chunks of 128 rows
  on SBUF partitions.
- Host folds mask*scale_sp into coef and packs per-input-group (q = 16
  input dims) weight matrices so the spline contraction, the SiLU base
  term, and the final sum over input dims are all TensorEngine matmuls.
- DVE computes exact Cox-de Boor bases (same op order/rounding as the
  reference, support-pruned to the nonzero j range).
- preacts is written by a broadcast-AP DMA straight from the x tile.
"""

import sys

sys.path.insert(0, "/opt/trn_rl_repo")

import numpy as np

import concourse.bass as bass
import concourse.tile as tile
from concourse import mybir
from concourse.bass_utils import run_bass_kernel_spmd
from concourse.masks import make_identity
from concourse.vector_clock import VectorClock
from concourse.tile_scheduler import N_PROCS
import bass_rust as _bass_rust

F32 = mybir.dt.float32
ALU = mybir.AluOpType

IN_DIM = 64
OUT_DIM = 64
NUM = 5
K = 3
SIZE = IN_DIM * OUT_DIM
BATCH = 4096
NCORES = 8
BSH = BATCH // NCORES          # 512 batch rows per core
NCH = BSH // 128               # 4 chunks of 128 rows
NI = NCH * IN_DIM              # 256 = free width of the x tile
NQ = 4                         # input-dim groups of 16
QW = IN_DIM // NQ              # 16
NC_COEF = NUM + K              # 8 basis channels


def _patch_drain(chunk=1):
    # This container's walrus allows only ONE sync wait per Drain
    # (TPB_CTRL) instruction; split the TileContext tail-drain waits
    # across several drains.
    def _drain_and_barrier(self, tick_clock, wait_clock):
        gc = tick_clock.global_clock
        vals = [gc[p] for p in range(N_PROCS)]
        nonzero = [p for p, v in enumerate(vals) if v > 0]
        for i in range(0, len(nonzero), chunk):
            group = set(nonzero[i : i + chunk])
            partial = VectorClock(
                [vals[p] if p in group else 0 for p in range(N_PROCS)]
            )
            d = self.nc.sync.drain()
            wait_clock.add_sem_waits(
                d.ins, _bass_rust.ScopedClock({None: partial})
            )
        self.nc.all_engine_barrier()
        popped = self.nc._tile_sem_poison_stack.pop()
        assert popped is self._sem_poison
        self.nc.clear_and_free_semaphores(list(self.sems.allocated().values()))
        self.nc.all_engine_barrier()

    tile.TileContext._drain_and_barrier = _drain_and_barrier


_patch_drain()


def _split_waits_json(bir_json: bytes, limit: int = 1) -> bytes:
    """This walrus build allows only `limit` sync waits per instruction.
    Hoist excess waits onto injected EventSemaphore instructions that run
    immediately before the instruction on the same engine."""
    import json

    d = json.loads(bir_json)
    counter = [0]

    def fix_block(b):
        insts = b.get("instructions", [])
        out = []
        for ins in insts:
            si = ins.get("sync_info") or {}
            waits = si.get("on_wait") or []
            if len(waits) > limit:
                keep = waits[-limit:]
                extra = waits[:-limit]
                for w in extra:
                    counter[0] += 1
                    out.append(
                        {
                            "debug": ins.get("debug"),
                            "engine": ins["engine"],
                            "ins": [],
                            "name": f"wsplit_{counter[0]}",
                            "opcode": "EventSemaphore",
                            "outs": [],
                            "sync_info": {"on_update": [], "on_wait": [w]},
                        }
                    )
                si["on_wait"] = keep
            out.append(ins)
        b["instructions"] = out
        for sub in b.get("blocks") or []:
            fix_block(sub)

    for f in d["functions"]:
        for b in f["blocks"]:
            fix_block(b)
    return json.dumps(d).encode()


def _patch_compile():
    from concourse import bass_utils as _bu
    from concourse import bass2jax as _b2j

    if getattr(_bu.compile_bir_kernel, "_wsplit", False):
        return
    _orig = _bu.compile_bir_kernel

    def patched(bir_json, tmpdir, neff_name="file.neff"):
        return _orig(_split_waits_json(bir_json), tmpdir, neff_name)

    patched._wsplit = True
    _bu.compile_bir_kernel = patched
    _b2j.compile_bir_kernel = patched


_patch_compile()


def _knots_from_grid(grid_row):
    """Replicate the reference's f32 grid extension exactly."""
    g = grid_row.astype(np.float32)
    h = np.float32((g[-1] - g[0]) / np.float32(g.shape[0] - 1))
    for _ in range(K):
        g = np.concatenate([g[:1] - h, g, g[-1:] + h]).astype(np.float32)
    return g  # 12 knots


def _lr_consts(g):
    """Per-level (d=1..3) left/right affine constants, f32.

    left_j(x)  = (x - g[j]) * invl[d][j]
    right_j(x) = (x - g[j+d+1]) * ninvr[d][j]   (= (g[j+d+1]-x)/denom)
    """
    one = np.float32(1.0)
    invl, ninvr, tl, tr = {}, {}, {}, {}
    for d in range(1, K + 1):
        nj = len(g) - d - 1
        invl[d] = np.array(
            [one / np.float32(g[j + d] - g[j]) for j in range(nj)], np.float32
        )
        ninvr[d] = np.array(
            [-(one / np.float32(g[j + d + 1] - g[j + 1])) for j in range(nj)],
            np.float32,
        )
        tl[d] = g[:nj].copy()
        tr[d] = g[d + 1 : d + 1 + nj].copy()
    return tl, invl, tr, ninvr


def _build_program():
    nc = bass.Bass("TRN2")

    xs = nc.dram_tensor("xs", [128, NCH, IN_DIM], F32, kind="ExternalInput")
    wraw = nc.dram_tensor("wraw", [NQ, 128, 1024], F32, kind="ExternalInput")
    wsc = nc.dram_tensor("wsc", [NQ, 128, 1024], F32, kind="ExternalInput")
    wb = nc.dram_tensor("wb", [NQ, QW, 1024], F32, kind="ExternalInput")
    wosp = nc.dram_tensor("wosp", [NQ, 128, OUT_DIM], F32, kind="ExternalInput")
    wob = nc.dram_tensor("wob", [NQ, QW, OUT_DIM], F32, kind="ExternalInput")
    kn = nc.dram_tensor("kn", [1, 64], F32, kind="ExternalInput")  # knot consts

    out_d = nc.dram_tensor("out", [NCH, 128, OUT_DIM], F32, kind="ExternalOutput")
    pre_d = nc.dram_tensor("pre", [NCH, 128, SIZE], F32, kind="ExternalOutput")
    pa_d = nc.dram_tensor("pa", [NCH, 128, SIZE], F32, kind="ExternalOutput")
    psp_d = nc.dram_tensor("psp", [NCH, 128, SIZE], F32, kind="ExternalOutput")

    # knot constants baked on host; slot layout in kn:
    #   [0:12] knots, then per level d: tl, invl, tr, ninvr packed later.
    # (scalars are compiled as immediates; kn kept for debugging only)

    with tile.TileContext(nc) as tc:
        with (
            tc.tile_pool(name="persist", bufs=1) as persist,
            tc.tile_pool(name="lr", bufs=2) as lrpool,
            tc.tile_pool(name="bt", bufs=8) as btpool,
            tc.tile_pool(name="osb", bufs=2) as osb,
            tc.tile_pool(name="psmm", bufs=5, space="PSUM") as psmm,
            tc.tile_pool(name="pstr", bufs=2, space="PSUM") as pstr,
            tc.tile_pool(name="psout", bufs=1, space="PSUM") as psout,
        ):
            ident = persist.tile([128, 128], F32)
            make_identity(nc, ident)

            x_t = persist.tile([128, NCH, IN_DIM], F32)
            nc.sync.dma_start(out=x_t[:], in_=xs[:, :, :])

            # preacts: pure broadcast of x over the out_dim axis.
            for n in range(NCH):
                nc.gpsimd.dma_start(
                    out=pre_d[n],
                    in_=x_t[:, n, None, :].broadcast_to([128, OUT_DIM, IN_DIM]),
                )

            # ---- weight loads ----
            w_raw_s = persist.tile([128, NQ, 1024], F32)
            w_sc_s = persist.tile([128, NQ, 1024], F32)
            w_b_s = persist.tile([QW, NQ, 1024], F32)
            w_osp_s = persist.tile([128, NQ, OUT_DIM], F32)
            w_ob_s = persist.tile([QW, NQ, OUT_DIM], F32)
            nc.sync.dma_start(out=w_raw_s[:], in_=wraw.transpose([1, 0, 2]))
            nc.sync.dma_start(out=w_sc_s[:], in_=wsc.transpose([1, 0, 2]))
            nc.scalar.dma_start(out=w_b_s[:], in_=wb.transpose([1, 0, 2]))
            nc.scalar.dma_start(out=w_osp_s[:], in_=wosp.transpose([1, 0, 2]))
            nc.scalar.dma_start(out=w_ob_s[:], in_=wob.transpose([1, 0, 2]))

            # ---- exact Cox-de Boor bases on [128, NCH*IN_DIM] ----
            g = _build_program.knots
            tl, invl, tr, ninvr = _build_program.lr

            # j/c channel INNERMOST so per-(n,q) transpose slices are
            # contiguous [128, 16*8] (matmul RHS wants one free dim).
            xflat = x_t[:].rearrange("p n i -> p (n i)")

            ge = persist.tile([128, NI, 6], F32)
            for jj, j in enumerate(range(3, 9)):
                nc.vector.tensor_scalar(
                    out=ge[:, :, jj], in0=xflat,
                    scalar1=float(g[j]), scalar2=None, op0=ALU.is_ge,
                )
            b0 = persist.tile([128, NI, 5], F32)  # j = 3..7
            nc.vector.tensor_sub(b0[:], ge[:, :, 0:5], ge[:, :, 1:6])

            def level(bprev, jlo_prev, jlo, njout, d):
                """bprev holds B^{d-1} for j = jlo_prev .. ; returns B^d tile
                for j = jlo .. jlo+njout-1 (support-pruned)."""
                bnew = persist.tile([128, NI, njout], F32, tag=f"b{d}")
                lbuf = lrpool.tile([128, NI, njout], F32, tag="lbuf")
                rbuf = lrpool.tile([128, NI, njout], F32, tag="rbuf")
                for k in range(njout):
                    j = jlo + k
                    if k > 0:  # left factor needed (B^{d-1}_j nonzero)
                        nc.vector.tensor_scalar(
                            out=lbuf[:, :, k], in0=xflat,
                            scalar1=float(tl[d][j]), scalar2=float(invl[d][j]),
                            op0=ALU.subtract, op1=ALU.mult,
                        )
                    if k < njout - 1:  # right factor
                        nc.vector.tensor_scalar(
                            out=rbuf[:, :, k], in0=xflat,
                            scalar1=float(tr[d][j]), scalar2=float(ninvr[d][j]),
                            op0=ALU.subtract, op1=ALU.mult,
                        )
                # edge j = jlo: right-only; edge j = jlo+njout-1: left-only
                nc.vector.tensor_mul(bnew[:, :, 0], rbuf[:, :, 0], bprev[:, :, 0])
                nc.vector.tensor_mul(
                    bnew[:, :, njout - 1],
                    lbuf[:, :, njout - 1],
                    bprev[:, :, njout - 2],
                )
                ni_ = njout - 2
                nc.vector.tensor_mul(
                    lbuf[:, :, 1 : 1 + ni_],
                    lbuf[:, :, 1 : 1 + ni_],
                    bprev[:, :, 0:ni_],
                )
                nc.vector.tensor_mul(
                    rbuf[:, :, 1 : 1 + ni_],
                    rbuf[:, :, 1 : 1 + ni_],
                    bprev[:, :, 1 : 1 + ni_],
                )
                nc.vector.tensor_add(
                    bnew[:, :, 1 : 1 + ni_],
                    lbuf[:, :, 1 : 1 + ni_],
                    rbuf[:, :, 1 : 1 + ni_],
                )
                return bnew

            b1 = level(b0, 3, 2, 6, 1)   # j = 2..7
            b2 = level(b1, 2, 1, 7, 2)   # j = 1..7
            b3 = level(b2, 1, 0, 8, 3)   # j = 0..7 -> [128, (n i), 8]

            b3v = b3[:].rearrange("p (n i) c -> p n i c", n=NCH)

            # silu(x)^T, one [16, 128] tile per (q, n) so every matmul lhsT
            # starts at partition base 0 (PE quadrant rule).
            silu_sb = persist.tile([QW, NQ, NCH, 128], F32)

            out_sb_pool = osb
            for n in range(NCH):
                for q in range(NQ):
                    xt_ps = pstr.tile([QW, 128], F32, tag="tr")
                    nc.tensor.transpose(
                        xt_ps[:], x_t[:, n, q * QW : (q + 1) * QW], ident[:]
                    )
                    nc.scalar.activation(
                        out=silu_sb[:, q, n, :], in_=xt_ps[:],
                        func=mybir.ActivationFunctionType.Silu,
                    )

            for n in range(NCH):
                # transpose bases: per q, [128b, (ii c)] -> [(ii c), 128b]
                bts = []
                for q in range(NQ):
                    src = b3v[:, n, q * QW : (q + 1) * QW, :]
                    bt_ps = pstr.tile([128, 128], F32, tag="tr")
                    nc.tensor.transpose(bt_ps[:], src, ident[:])
                    bt_sb = btpool.tile([128, 128], F32, tag="btsb")
                    nc.scalar.copy(out=bt_sb[:], in_=bt_ps[:])
                    bts.append(bt_sb)

                psp_sb = osb.tile([128, OUT_DIM, IN_DIM], F32, tag="psp")
                pa_sb = osb.tile([128, OUT_DIM, IN_DIM], F32, tag="pa")
                out_ps = psout.tile([128, OUT_DIM], F32, tag="outps")

                eng_i = 0
                for q in range(NQ):
                    for h in range(2):
                        cols = slice(h * 512, (h + 1) * 512)
                        osl = slice(32 * h, 32 * h + 32)
                        qsl = slice(q * QW, (q + 1) * QW)
                        # postspline
                        mm = psmm.tile([128, 32, QW], F32, tag="mm")
                        nc.tensor.matmul(
                            mm[:], bts[q][:], w_raw_s[:, q, cols],
                            start=True, stop=True,
                        )
                        # postacts = scaled spline + silu base
                        mm2 = psmm.tile([128, 32, QW], F32, tag="mm")
                        nc.tensor.matmul(
                            mm2[:], bts[q][:], w_sc_s[:, q, cols],
                            start=True, stop=False,
                        )
                        nc.tensor.matmul(
                            mm2[:], silu_sb[:, q, n, :], w_b_s[:, q, cols],
                            start=False, stop=True,
                        )
                        if eng_i % 2 == 0:
                            nc.scalar.copy(out=psp_sb[:, osl, qsl], in_=mm[:])
                            nc.vector.tensor_copy(out=pa_sb[:, osl, qsl], in_=mm2[:])
                        else:
                            nc.vector.tensor_copy(out=psp_sb[:, osl, qsl], in_=mm[:])
                            nc.scalar.copy(out=pa_sb[:, osl, qsl], in_=mm2[:])
                        eng_i += 1

                # out = sum_i postacts: dense per-q matmuls, accumulate
                for q in range(NQ):
                    qsl = slice(q * QW, (q + 1) * QW)
                    nc.tensor.matmul(
                        out_ps[:], bts[q][:], w_osp_s[:, q, :],
                        start=(q == 0), stop=False,
                    )
                    nc.tensor.matmul(
                        out_ps[:], silu_sb[:, q, n, :], w_ob_s[:, q, :],
                        start=False, stop=(q == NQ - 1),
                    )
                o_sb = out_sb_pool.tile([128, OUT_DIM], F32, tag="osb")
                nc.scalar.copy(out=o_sb[:], in_=out_ps[:])

                nc.sync.dma_start(
                    out=psp_d[n], in_=psp_sb[:].rearrange("p a b -> p (a b)")
                )
                nc.scalar.dma_start(
                    out=pa_d[n], in_=pa_sb[:].rearrange("p a b -> p (a b)")
                )
                nc.sync.dma_start(out=out_d[n], in_=o_sb[:])

    return nc


def _prep_host(x, grid, coef, scale_base, scale_sp, mask):
    x = np.asarray(x, np.float32)
    grid = np.asarray(grid, np.float32)
    coef = np.asarray(coef, np.float32)
    scale_base = np.asarray(scale_base, np.float32)
    scale_sp = np.asarray(scale_sp, np.float32)
    mask = np.asarray(mask, np.float32)

    g = _knots_from_grid(grid[0])
    _build_program.knots = g
    _build_program.lr = _lr_consts(g)

    mss = (mask * scale_sp).astype(np.float32)
    msb = (mask * scale_base).astype(np.float32)
    coef_sc = (coef * mss[:, None]).astype(np.float32)

    coef_r = coef.reshape(OUT_DIM, IN_DIM, NC_COEF)
    coef_sc_r = coef_sc.reshape(OUT_DIM, IN_DIM, NC_COEF)
    msb_r = msb.reshape(OUT_DIM, IN_DIM)

    wraw = np.zeros((NQ, QW, NC_COEF, OUT_DIM, QW), np.float32)
    wsc = np.zeros_like(wraw)
    wb = np.zeros((NQ, QW, OUT_DIM, QW), np.float32)
    wosp = np.zeros((NQ, QW, NC_COEF, OUT_DIM), np.float32)
    wob = np.zeros((NQ, QW, OUT_DIM), np.float32)
    for q in range(NQ):
        for ii in range(QW):
            i = q * QW + ii
            wraw[q, ii, :, :, ii] = coef_r[:, i, :].T
            wsc[q, ii, :, :, ii] = coef_sc_r[:, i, :].T
            wb[q, ii, :, ii] = msb_r[:, i]
            wosp[q, ii, :, :] = coef_sc_r[:, i, :].T
            wob[q, ii, :] = msb_r[:, i]
    wraw = wraw.reshape(NQ, 128, 1024)
    wsc = wsc.reshape(NQ, 128, 1024)
    wb = wb.reshape(NQ, QW, 1024)
    wosp = wosp.reshape(NQ, 128, OUT_DIM)
    wob = wob.reshape(NQ, QW, OUT_DIM)

    kn = np.zeros((1, 64), np.float32)
    kn[0, :12] = g

    in_maps = []
    for c in range(NCORES):
        xsh = x[c * BSH : (c + 1) * BSH]  # [512, 64]
        xst = np.ascontiguousarray(
            xsh.reshape(NCH, 128, IN_DIM).transpose(1, 0, 2)
        )  # [128, NCH, 64]
        in_maps.append(
            dict(xs=xst, wraw=wraw, wsc=wsc, wb=wb, wosp=wosp, wob=wob, kn=kn)
        )
    return in_maps


_PROGRAM_CACHE = {}


def kernel(x, grid, coef, scale_base, scale_sp, mask, _trace=False):
    in_maps = _prep_host(x, grid, coef, scale_base, scale_sp, mask)
    key = "prog"
    if key not in _PROGRAM_CACHE:
        _PROGRAM_CACHE[key] = _build_program()
    nc = _PROGRAM_CACHE[key]

    res = run_bass_kernel_spmd(
        nc, in_maps, core_ids=list(range(NCORES)), trace=_trace
    )
    kernel.last_result = res

    outs, pres, pas, psps = [], [], [], []
    for c in range(NCORES):
        r = res.results[c]
        # [NCH, 128, ...] with b_local = n*128 + p -> reshape direct
        outs.append(r["out"].reshape(BSH, OUT_DIM))
        pres.append(r["pre"].reshape(BSH, OUT_DIM, IN_DIM))
        pas.append(r["pa"].reshape(BSH, OUT_DIM, IN_DIM))
        psps.append(r["psp"].reshape(BSH, OUT_DIM, IN_DIM))
    out = np.concatenate(outs, 0)
    preacts = np.concatenate(pres, 0)
    postacts = np.concatenate(pas, 0)
    postspline = np.concatenate(psps, 0)
    return (out, preacts, postacts, postspline)
